# revision 1
# baseline (speedup 1.0000x reference)
"""Causal single-head attention (B=4, S=2048, D=1024) on 8 Trainium2 NeuronCores.

Sharding: core = (batch, parity). Each batch's 4 query-chunks of 512 are split
{0,3} / {1,2} across its two cores so causal work balances exactly (each core
processes one 8-k-block chunk and one 16-k-block chunk). K/V are computed from
the full batch sequence on every core (replicated projection — cheaper than
exchanging K/V between cores).

Device algorithm per core (all matmuls bf16 operands, fp32 PSUM accumulation):
  kT[o,s] = WkT.T @ xT      (scores want K transposed: d on partitions)
  v[s,o]  = xT.T @ WvT
  qT[o,s] = (WqT/32).T @ xTq   (softmax scale folded into Wq on host — exact)
  scores_T[sk,sq] = kT_blk.T @ qT_chunk   -> PSUM
  p = exp(scores_T) * mask   (no max-subtraction: logits are O(1) by
                              construction; masked entries multiply to 0)
  out[sq,o] = sum_blk p_blk.T @ v_blk ; den[sq] = sum_blk p_blk.T @ ones
  out = out * (1/den)

The transposed-scores layout keeps the softmax reduction on the PE (ones
matmul) and feeds attn@V without transposing p.
"""

import sys

if "/opt/trn_rl_repo" not in sys.path:
    sys.path.insert(0, "/opt/trn_rl_repo")

import numpy as np
import ml_dtypes

import concourse.mybir as mybir
import concourse.tile as tile
from concourse import bacc
from concourse.bass_utils import run_bass_kernel_spmd

bf16 = ml_dtypes.bfloat16

B, S, D = 4, 2048, 1024
CH = 512            # projection column-chunk width
QC = 256            # attention query-chunk width
BLK = 128           # key-block
# Per-core schedule: 4 query-chunks of 256, processed with a fixed padded
# k-block count (4,8,12,16). Host assigns real chunks sorted by causal depth
# so padding waste is exactly 4 blocks/core; masks (data) encode reality.
SCHED = (4, 8, 12, 16)
MASK_BASE = (0, 4, 12, 24)
NMASK = sum(SCHED)  # 40
DT8 = D // 128      # contraction tiles
N_CORES = 8
DT_BF = mybir.dt.bfloat16
DT_F32 = mybir.dt.float32

_NC_CACHE = {}


def _emit(tc, xT, xTq, wqT, wkT, wvT, msk, out):
    nc = tc.nc
    Exp = mybir.ActivationFunctionType.Exp

    with (
        tc.tile_pool(name="const", bufs=1) as constp,
        tc.tile_pool(name="kv", bufs=1) as kv,
    ):
        ones = constp.tile([128, 1], DT_BF, tag="ones", name="ones")
        nc.vector.memset(ones, 1.0)

        kT_t = [
            [kv.tile([128, CH], DT_BF, tag=f"kT{i}_{sc}", name=f"kT{i}_{sc}")
             for sc in range(S // CH)]
            for i in range(DT8)
        ]
        v_t = [kv.tile([128, D], DT_BF, tag=f"v{i}", name=f"v{i}") for i in range(S // 128)]
        qT_t = [kv.tile([128, 2 * CH], DT_BF, tag=f"qT{i}", name=f"qT{i}") for i in range(DT8)]

        with (
            tc.tile_pool(name="xw", bufs=1) as xw,
            tc.tile_pool(name="proj_ps", bufs=2, space="PSUM") as pps,
        ):
            # xT / wv split into per-chunk tiles: Tile dependency tracking is
            # tile-granular, so the first projection group must only wait for
            # its own 2MB of DMA instead of all 6MB.
            xT_t = [
                [xw.tile([128, CH], DT_BF, tag=f"xT{i}_{sc}", name=f"xTs{i}_{sc}")
                 for sc in range(S // CH)]
                for i in range(DT8)
            ]
            wv_t = [
                [xw.tile([128, CH], DT_BF, tag=f"wv{i}_{oc}", name=f"wv{i}_{oc}")
                 for oc in range(D // CH)]
                for i in range(DT8)
            ]
            wk_t = [xw.tile([128, D], DT_BF, tag=f"wk{i}", name=f"wk{i}") for i in range(DT8)]
            wq_t = [xw.tile([128, D], DT_BF, tag=f"wq{i}", name=f"wq{i}") for i in range(DT8)]
            xq_t = [
                [xw.tile([128, CH], DT_BF, tag=f"xq{i}_{sc}", name=f"xq{i}_{sc}")
                 for sc in range(2)]
                for i in range(DT8)
            ]

            # DMAs in consumption order so the first matmuls start early:
            # wv o-chunk 0 (1MB) + xT s-chunk 0 (1MB) unblocks the first
            # v-projection group after ~2MB instead of 6MB.
            # Alternate HWDGE (sync) and SWDGE (gpsimd) so input loads use
            # both DMA queue pools — HWDGE alone tops out around half the
            # per-core HBM bandwidth.
            def _dma(n, out, in_):
                eng = nc.sync if n % 2 == 0 else nc.gpsimd
                eng.dma_start(out=out, in_=in_)

            for oc in range(D // CH):
                for i in range(DT8):
                    _dma(i, wv_t[i][oc],
                         wvT[128 * i : 128 * (i + 1), CH * oc : CH * (oc + 1)])
                for i in range(DT8):
                    _dma(i + 1, xT_t[i][oc],
                         xT[128 * i : 128 * (i + 1), CH * oc : CH * (oc + 1)])
            for sc in range(2, S // CH):
                for i in range(DT8):
                    _dma(i + sc, xT_t[i][sc],
                         xT[128 * i : 128 * (i + 1), CH * sc : CH * (sc + 1)])
            for i in range(DT8):
                _dma(i, wq_t[i], wqT[128 * i : 128 * (i + 1), :])
                _dma(i + 1, xq_t[i][0], xTq[128 * i : 128 * (i + 1), 0:CH])
                _dma(i, xq_t[i][1], xTq[128 * i : 128 * (i + 1), CH : 2 * CH])
            for i in range(DT8):
                _dma(i, wk_t[i], wkT[128 * i : 128 * (i + 1), :])

            # v projection (st-outer consumes xT column chunks in DMA
            # arrival order): v[s,o] += xT[i,s].T @ WvT[i,o]
            for st in range(S // 128):
                sc, so = st // 4, (st % 4) * 128
                for oc in range(D // CH):
                    ps = pps.tile([128, CH], DT_F32, tag="pps", name="pps")
                    for i in range(DT8):
                        nc.tensor.matmul(
                            ps,
                            lhsT=xT_t[i][sc][:, so : so + 128],
                            rhs=wv_t[i][oc],
                            start=(i == 0),
                            stop=(i == DT8 - 1),
                        )
                    nc.scalar.copy(out=v_t[st][:, CH * oc : CH * (oc + 1)], in_=ps)
            # qT projection (Wq pre-scaled by 1/32 on host) — before kT so the
            # first scores group is not gated on the very last projection copy
            for ot in range(DT8):
                for sc in range(2):
                    ps = pps.tile([128, CH], DT_F32, tag="pps", name="pps")
                    for i in range(DT8):
                        nc.tensor.matmul(
                            ps,
                            lhsT=wq_t[i][:, 128 * ot : 128 * (ot + 1)],
                            rhs=xq_t[i][sc],
                            start=(i == 0),
                            stop=(i == DT8 - 1),
                        )
                    nc.scalar.copy(out=qT_t[ot][:, CH * sc : CH * (sc + 1)], in_=ps)
            # kT projection: kT[o,s] += WkT[i,o].T @ xT[i,s]
            for sc in range(S // CH):
                for ot in range(DT8):
                    ps = pps.tile([128, CH], DT_F32, tag="pps", name="pps")
                    for i in range(DT8):
                        nc.tensor.matmul(
                            ps,
                            lhsT=wk_t[i][:, 128 * ot : 128 * (ot + 1)],
                            rhs=xT_t[i][sc],
                            start=(i == 0),
                            stop=(i == DT8 - 1),
                        )
                    nc.vector.tensor_copy(out=kT_t[ot][sc], in_=ps)

        # ---- attention ----
        with (
            tc.tile_pool(name="attn_sb", bufs=1) as asb,
            tc.tile_pool(name="mask_sb", bufs=4) as msb,
            tc.tile_pool(name="outs_sb", bufs=2) as osb,
            tc.tile_pool(name="score_ps", bufs=2, space="PSUM") as sps,
            tc.tile_pool(name="out_ps", bufs=2, space="PSUM") as ops,
            tc.tile_pool(name="den_ps", bufs=2, space="PSUM") as dps,
        ):
            p_t = {}
            for L in range(4):
                for b in range(SCHED[L]):
                    ps = sps.tile([128, QC], DT_F32, tag="sps", name="sps")
                    for i in range(DT8):
                        nc.tensor.matmul(
                            ps,
                            lhsT=kT_t[i][b // 4][:, BLK * (b % 4) : BLK * (b % 4 + 1)],
                            rhs=qT_t[i][:, QC * L : QC * (L + 1)],
                            start=(i == 0),
                            stop=(i == DT8 - 1),
                        )
                    m = msb.tile([128, QC], DT_BF, tag="mask", name="mask")
                    nc.sync.dma_start(out=m, in_=msk[MASK_BASE[L] + b])
                    es = msb.tile([128, QC], DT_BF, tag="es", name="es")
                    nc.scalar.activation(es, ps, Exp)
                    p = asb.tile([128, QC], DT_BF, tag=f"p{L}_{b}", name=f"p{L}_{b}")
                    nc.vector.tensor_mul(p, es, m)
                    p_t[(L, b)] = p

            for L in range(4):
                nblk = SCHED[L]
                for sqt in range(QC // 128):
                    po = ops.tile([128, D], DT_F32, tag="po", name="po")
                    pd = dps.tile([128, 1], DT_F32, tag="pd", name="pd")
                    for b in range(nblk):
                        pt = p_t[(L, b)][:, 128 * sqt : 128 * (sqt + 1)]
                        nc.tensor.matmul(
                            po[:, 0:CH], lhsT=pt, rhs=v_t[b][:, 0:CH],
                            start=(b == 0), stop=(b == nblk - 1),
                            skip_group_check=True,
                        )
                        nc.tensor.matmul(
                            po[:, CH:D], lhsT=pt, rhs=v_t[b][:, CH:D],
                            start=(b == 0), stop=(b == nblk - 1),
                            skip_group_check=True,
                        )
                        nc.tensor.matmul(
                            pd, lhsT=pt, rhs=ones,
                            start=(b == 0), stop=(b == nblk - 1),
                            skip_group_check=True,
                        )
                    r = osb.tile([128, 1], DT_F32, tag="r", name="r")
                    nc.vector.reciprocal(r, pd)
                    o = osb.tile([128, D], DT_F32, tag="osb", name="osb")
                    nc.vector.tensor_scalar_mul(o, po, r)
                    nc.sync.dma_start(
                        out=out[QC * L + 128 * sqt : QC * L + 128 * (sqt + 1), :],
                        in_=o,
                    )


def build_program():
    nc = bacc.Bacc(
        "TRN2",
        target_bir_lowering=False,
        debug=False,
        enable_asserts=False,
        num_devices=N_CORES,
    )
    xT = nc.dram_tensor("xT", [D, S], DT_BF, kind="ExternalInput").ap()
    xTq = nc.dram_tensor("xTq", [D, 2 * CH], DT_BF, kind="ExternalInput").ap()
    wqT = nc.dram_tensor("wqT", [D, D], DT_BF, kind="ExternalInput").ap()
    wkT = nc.dram_tensor("wkT", [D, D], DT_BF, kind="ExternalInput").ap()
    wvT = nc.dram_tensor("wvT", [D, D], DT_BF, kind="ExternalInput").ap()
    msk = nc.dram_tensor("msk", [NMASK, BLK, QC], DT_BF, kind="ExternalInput").ap()
    out = nc.dram_tensor("out", [2 * CH, D], DT_F32, kind="ExternalOutput").ap()
    with tile.TileContext(nc) as tc:
        _emit(tc, xT, xTq, wqT, wkT, wvT, msk, out)
    nc.compile()
    return nc


def get_program():
    if "nc" not in _NC_CACHE:
        _NC_CACHE["nc"] = build_program()
    return _NC_CACHE["nc"]


def _chunks_for(core):
    """Per-core 256-wide query chunks, L-ordered to match SCHED=(4,8,12,16).
    Real causal k-block need: chunk j -> 2(j+1)."""
    return [0, 3, 4, 7] if core % 2 == 0 else [1, 2, 5, 6]


def _build_masks(chunks):
    """[40,128,256] in {0,1}: allowed(sk=128*blk+p, sq=256*j+c) = sk <= sq.
    Padding blocks beyond a chunk's real causal depth come out all-zero."""
    m = np.zeros((NMASK, BLK, QC), np.float32)
    p = np.arange(BLK)[:, None]
    c = np.arange(QC)[None, :]
    for L, j in enumerate(chunks):
        for b in range(SCHED[L]):
            m[MASK_BASE[L] + b] = BLK * b + p <= QC * j + c
    return m.astype(bf16)


def build_in_maps(x, Wq, Wk, Wv):
    wq = np.ascontiguousarray(Wq.T.astype(np.float32) / 32.0).astype(bf16)
    wk = np.ascontiguousarray(Wk.T).astype(bf16)
    wv = np.ascontiguousarray(Wv.T).astype(bf16)
    masks = {par: _build_masks(_chunks_for(par)) for par in (0, 1)}
    in_maps = []
    for core in range(N_CORES):
        b = core // 2
        chunks = _chunks_for(core)
        xTb = np.ascontiguousarray(x[b].T).astype(bf16)  # [D, S]
        xq = np.ascontiguousarray(
            np.concatenate(
                [xTb[:, QC * j : QC * (j + 1)] for j in chunks], axis=1
            )
        )
        in_maps.append(
            {"xT": xTb, "xTq": xq, "wqT": wq, "wkT": wk, "wvT": wv,
             "msk": masks[core % 2]}
        )
    return in_maps


def assemble_output(results):
    out = np.zeros((B, S, D), np.float32)
    for core in range(N_CORES):
        b = core // 2
        for L, j in enumerate(_chunks_for(core)):
            out[b, QC * j : QC * (j + 1)] = results[core]["out"][QC * L : QC * (L + 1)]
    return out


def kernel(x, Wq, Wk, Wv):
    x = np.asarray(x, np.float32)
    nc = get_program()
    in_maps = build_in_maps(x, np.asarray(Wq, np.float32),
                            np.asarray(Wk, np.float32), np.asarray(Wv, np.float32))
    res = run_bass_kernel_spmd(nc, in_maps, core_ids=list(range(N_CORES)))
    return assemble_output(res.results)



# revision 6
# speedup vs baseline: 1.4938x; 1.4938x over previous
"""Causal single-head attention (B=4, S=2048, D=1024) on 8 Trainium2 NeuronCores.

Sharding: core = (batch, parity). Each batch's 8 query-chunks of 256 are split
{0,3,4,7} / {1,2,5,6} across its two cores so causal work balances exactly.

Algebraic restructure vs the naive QKV form (saves 40% of PE work):
  scores = (x Wq^T)(x Wk^T)^T = x (Wq^T Wk) x^T = x M x^T
with M = Wq^T Wk / 32 precomputed on the HOST — the K projection disappears
and scores contract q' = x_q M directly against raw x^T. Likewise
  out = P v = P (x Wv^T) = (P x) Wv^T
so the V projection disappears and P contracts against raw x; the small
(Px) Wv^T projection runs once per query block. Per-core PE work drops from
8.05G to 4.83G MACs with no cross-core communication.

Device algorithm per core (bf16 operands, fp32 PSUM):
  q'T[j,q]   = M_t.T @ xTq          (1.07G)  per L-chunk
  s_T[sk,sq] = xT_blk.T @ q'T       (1.34G)  logits directly (1/32 inside M)
  p = exp(s_T) * mask               (no max-subtraction: logits are O(1))
  PxT[i,sq]  = xS_blk.T @ p_blk     (1.34G)  den[sq] = p_blk.T @ ones
  out[sq,o]  = (PxT.T @ WvT) / den  (1.07G)
"""

import sys

if "/opt/trn_rl_repo" not in sys.path:
    sys.path.insert(0, "/opt/trn_rl_repo")

import numpy as np
import ml_dtypes

import concourse.mybir as mybir
import concourse.tile as tile
from concourse import bacc
from concourse.bass_utils import run_bass_kernel_spmd

bf16 = ml_dtypes.bfloat16

B, S, D = 4, 2048, 1024
CH = 512            # xT column-chunk width (SBUF tile granularity)
QC = 256            # query-chunk width
BLK = 128           # key-block
# Per-core schedule: 4 query-chunks of 256, processed with a fixed padded
# k-block count (4,8,12,16). Host assigns real chunks sorted by causal depth
# so padding waste is exactly 4 blocks/core; masks (data) encode reality.
SCHED = (4, 8, 12, 16)
MASK_BASE = (0, 4, 12, 24)
NMASK = sum(SCHED)  # 40
DT8 = D // 128      # contraction tiles
N_CORES = 8
DT_BF = mybir.dt.bfloat16
DT_F32 = mybir.dt.float32

_NC_CACHE = {}


def _emit(tc, xT, xS, m, wvT, msk, out):
    nc = tc.nc
    Exp = mybir.ActivationFunctionType.Exp

    with (
        tc.tile_pool(name="const", bufs=1) as constp,
        tc.tile_pool(name="sb", bufs=1) as sb,
        tc.tile_pool(name="outs_sb", bufs=2) as osb,
        tc.tile_pool(name="sps", bufs=4, space="PSUM") as sps,
        tc.tile_pool(name="ops", bufs=1, space="PSUM") as ops,
        tc.tile_pool(name="dps", bufs=2, space="PSUM") as dps,
    ):
        ones = constp.tile([128, 1], DT_BF, tag="ones", name="ones")
        nc.vector.memset(ones, 1.0)

        xT_t = [
            [sb.tile([128, CH], DT_BF, tag=f"xT{i}_{sc}", name=f"xT{i}_{sc}")
             for sc in range(S // CH)]
            for i in range(DT8)
        ]
        xS_t = [sb.tile([128, D], DT_BF, tag=f"xS{b}", name=f"xS{b}")
                for b in range(S // 128)]
        m_t = [sb.tile([128, D], DT_BF, tag=f"m{i}", name=f"m{i}") for i in range(DT8)]
        wv_t = [sb.tile([128, D], DT_BF, tag=f"wv{i}", name=f"wv{i}") for i in range(DT8)]
        qT_t = [sb.tile([128, 4 * QC], DT_BF, tag=f"qT{i}", name=f"qT{i}") for i in range(DT8)]
        msk_t = [sb.tile([128, QC], DT_BF, tag=f"msk{n}", name=f"msk{n}")
                 for n in range(NMASK)]
        p_t = {}
        px_t = {}
        for L in range(4):
            for b in range(SCHED[L]):
                p_t[(L, b)] = sb.tile([128, QC], DT_BF, tag=f"p{L}_{b}", name=f"p{L}_{b}")
            for i in range(DT8):
                px_t[(L, i)] = sb.tile([128, QC], DT_BF, tag=f"px{L}_{i}",
                                       name=f"px{L}_{i}")

        # DMAs in consumption order so the first matmuls start early.
        # Alternate HWDGE (sync) and SWDGE (gpsimd) so input loads use both
        # DMA queue pools — HWDGE alone tops out around half the per-core
        # HBM bandwidth.
        def _dma(n, dst, src):
            eng = nc.sync if n % 2 == 0 else nc.gpsimd
            eng.dma_start(out=dst, in_=src)

        def dma_m():
            for i in range(DT8):
                _dma(i, m_t[i], m[128 * i : 128 * (i + 1), :])

        def dma_xT(sc):
            for i in range(DT8):
                _dma(i + sc, xT_t[i][sc],
                     xT[128 * i : 128 * (i + 1), CH * sc : CH * (sc + 1)])

        def dma_xS(b0, b1):
            for b in range(b0, b1):
                _dma(b, xS_t[b], xS[128 * b : 128 * (b + 1), :])

        def dma_msk(L):
            for b in range(SCHED[L]):
                _dma(b, msk_t[MASK_BASE[L] + b], msk[MASK_BASE[L] + b])

        def dma_wv():
            for i in range(DT8):
                _dma(i + 1, wv_t[i], wvT[128 * i : 128 * (i + 1), :])

        dma_m()
        dma_xT(0)
        dma_msk(0)
        dma_xS(0, 4)
        dma_xT(1)
        dma_msk(1)
        dma_xS(4, 8)
        dma_wv()
        dma_xT(2)
        dma_msk(2)
        dma_xS(8, 12)
        dma_xT(3)
        dma_msk(3)
        dma_xS(12, 16)

        # ---- PE pipeline ----
        def q_proj_emit(L, qoff):
            # qoff: column offset of chunk j_L inside xT sc-chunk L (0 or 256)
            for ot in range(DT8):
                ps = sps.tile([128, QC], DT_F32, tag="sps", name="sps")
                for i in range(DT8):
                    nc.tensor.matmul(
                        ps,
                        lhsT=m_t[i][:, 128 * ot : 128 * (ot + 1)],
                        rhs=xT_t[i][L][:, qoff : qoff + QC],
                        start=(i == 0),
                        stop=(i == DT8 - 1),
                    )
                nc.scalar.copy(out=qT_t[ot][:, QC * L : QC * (L + 1)], in_=ps)

        def scores_emit(L):
            for b in range(SCHED[L]):
                ps = sps.tile([128, QC], DT_F32, tag="sps", name="sps")
                for i in range(DT8):
                    nc.tensor.matmul(
                        ps,
                        lhsT=xT_t[i][b // 4][:, BLK * (b % 4) : BLK * (b % 4 + 1)],
                        rhs=qT_t[i][:, QC * L : QC * (L + 1)],
                        start=(i == 0),
                        stop=(i == DT8 - 1),
                    )
                es = osb.tile([128, QC], DT_BF, tag="es", name="es")
                nc.scalar.activation(es, ps, Exp)
                nc.vector.tensor_mul(p_t[(L, b)], es, msk_t[MASK_BASE[L] + b])

        def px_emit(L):
            # PxT[i, q] = sum_b xS[b][:, i-cols].T @ p[b]
            for i in range(DT8):
                ps = sps.tile([128, QC], DT_F32, tag="sps", name="sps")
                for b in range(SCHED[L]):
                    nc.tensor.matmul(
                        ps,
                        lhsT=xS_t[b][:, 128 * i : 128 * (i + 1)],
                        rhs=p_t[(L, b)],
                        start=(b == 0),
                        stop=(b == SCHED[L] - 1),
                    )
                if i % 2 == 0:
                    nc.scalar.copy(out=px_t[(L, i)], in_=ps)
                else:
                    nc.vector.tensor_copy(out=px_t[(L, i)], in_=ps)

        def den_out_emit(L):
            for sqt in range(QC // 128):
                pd = dps.tile([128, 1], DT_F32, tag="pd", name="pd")
                for b in range(SCHED[L]):
                    nc.tensor.matmul(
                        pd,
                        lhsT=p_t[(L, b)][:, 128 * sqt : 128 * (sqt + 1)],
                        rhs=ones,
                        start=(b == 0),
                        stop=(b == SCHED[L] - 1),
                    )
                po = ops.tile([128, D], DT_F32, tag="po", name="po")
                for i in range(DT8):
                    pxs = px_t[(L, i)][:, 128 * sqt : 128 * (sqt + 1)]
                    nc.tensor.matmul(
                        po[:, 0:CH], lhsT=pxs, rhs=wv_t[i][:, 0:CH],
                        start=(i == 0), stop=(i == DT8 - 1),
                        skip_group_check=True,
                    )
                    nc.tensor.matmul(
                        po[:, CH:D], lhsT=pxs, rhs=wv_t[i][:, CH:D],
                        start=(i == 0), stop=(i == DT8 - 1),
                        skip_group_check=True,
                    )
                r = osb.tile([128, 1], DT_F32, tag="r", name="r")
                nc.vector.reciprocal(r, pd)
                o = osb.tile([128, D], DT_BF, tag="osb", name="osb")
                nc.vector.tensor_scalar_mul(o, po, r)
                nc.sync.dma_start(
                    out=out[QC * L + 128 * sqt : QC * L + 128 * (sqt + 1), :],
                    in_=o,
                )

        # Chunk j occupies xT sc-chunk j//2 at column offset 256*(j%2); the
        # L-th chunk of either parity lives in sc-chunk L, but at offset
        # (0,256,0,256) for even cores {0,3,4,7} and (256,0,256,0) for odd
        # {1,2,5,6}. SPMD needs identical instructions, so the host applies a
        # key/seq permutation pos^256 for odd cores (to xT columns, xS rows,
        # and the masks' key coordinate) — after which QOFF below selects the
        # odd chunks too, and scores/Px stay consistent because xT, xS and
        # masks are permuted together (a summation reorder within each block
        # group).
        QOFF = (0, 256, 0, 256)

        q_proj_emit(0, QOFF[0])
        scores_emit(0)
        q_proj_emit(1, QOFF[1])
        scores_emit(1)
        px_emit(0)
        den_out_emit(0)
        q_proj_emit(2, QOFF[2])
        scores_emit(2)
        px_emit(1)
        den_out_emit(1)
        q_proj_emit(3, QOFF[3])
        scores_emit(3)
        px_emit(2)
        den_out_emit(2)
        px_emit(3)
        den_out_emit(3)


def build_program():
    nc = bacc.Bacc(
        "TRN2",
        target_bir_lowering=False,
        debug=False,
        enable_asserts=False,
        num_devices=N_CORES,
    )
    xT = nc.dram_tensor("xT", [D, S], DT_BF, kind="ExternalInput").ap()
    xS = nc.dram_tensor("xS", [S, D], DT_BF, kind="ExternalInput").ap()
    m = nc.dram_tensor("m", [D, D], DT_BF, kind="ExternalInput").ap()
    wvT = nc.dram_tensor("wvT", [D, D], DT_BF, kind="ExternalInput").ap()
    msk = nc.dram_tensor("msk", [NMASK, BLK, QC], DT_BF, kind="ExternalInput").ap()
    out = nc.dram_tensor("out", [4 * QC, D], DT_BF, kind="ExternalOutput").ap()
    with tile.TileContext(nc) as tc:
        _emit(tc, xT, xS, m, wvT, msk, out)
    nc.compile()
    return nc


def get_program():
    if "nc" not in _NC_CACHE:
        _NC_CACHE["nc"] = build_program()
    return _NC_CACHE["nc"]


def _chunks_for(core):
    """Per-core 256-wide query chunks, L-ordered to match SCHED=(4,8,12,16).
    Real causal k-block need: chunk j -> 2(j+1)."""
    return [0, 3, 4, 7] if core % 2 == 0 else [1, 2, 5, 6]


def _build_masks(chunks, permuted):
    """[40,128,256] in {0,1}: allowed iff actual_key <= actual_query, where
    for odd cores the key axis is permuted by pos^256 (see build_in_maps).
    Padding blocks beyond a chunk's real causal depth come out all-zero."""
    m = np.zeros((NMASK, BLK, QC), np.float32)
    p = np.arange(BLK)[:, None]
    c = np.arange(QC)[None, :]
    for L, j in enumerate(chunks):
        for b in range(SCHED[L]):
            sk = BLK * b + p
            if permuted:
                sk = sk ^ 256
            m[MASK_BASE[L] + b] = sk <= QC * j + c
    return m.astype(bf16)


def _perm256(a, axis):
    """Swap the 256-halves of every 512-chunk along `axis` (pos -> pos^256)."""
    sh = a.shape
    n = sh[axis]
    new_shape = sh[:axis] + (n // 512, 2, 256) + sh[axis + 1 :]
    return np.ascontiguousarray(
        np.flip(a.reshape(new_shape), axis=axis + 1).reshape(sh)
    )


def build_in_maps(x, Wq, Wk, Wv):
    Wq = np.asarray(Wq, np.float32)
    Wk = np.asarray(Wk, np.float32)
    Wv = np.asarray(Wv, np.float32)
    m = ((Wq.T @ Wk) / 32.0).astype(bf16)  # [d_in, d_in], softmax scale folded
    wv = np.ascontiguousarray(Wv.T).astype(bf16)
    masks = {par: _build_masks(_chunks_for(par), par == 1) for par in (0, 1)}
    in_maps = []
    for core in range(N_CORES):
        b = core // 2
        xb = np.asarray(x[b], np.float32).astype(bf16)  # [S, D]
        xTb = np.ascontiguousarray(np.asarray(x[b], np.float32).T).astype(bf16)
        if core % 2 == 1:
            # Sequence-permute by pos^256 so the kernel's fixed
            # QOFF=(0,256,0,256) selects odd chunks {1,2,5,6}; xT columns,
            # xS rows and mask key coordinates move together.
            xTb = _perm256(xTb, 1)
            xb = _perm256(xb, 0)
        in_maps.append(
            {"xT": xTb, "xS": xb, "m": m, "wvT": wv,
             "msk": masks[core % 2]}
        )
    return in_maps


def assemble_output(results):
    out = np.zeros((B, S, D), np.float32)
    for core in range(N_CORES):
        b = core // 2
        for L, j in enumerate(_chunks_for(core)):
            out[b, QC * j : QC * (j + 1)] = \
                results[core]["out"][QC * L : QC * (L + 1)].astype(np.float32)
    return out


def kernel(x, Wq, Wk, Wv):
    x = np.asarray(x, np.float32)
    nc = get_program()
    in_maps = build_in_maps(x, np.asarray(Wq, np.float32),
                            np.asarray(Wk, np.float32), np.asarray(Wv, np.float32))
    res = run_bass_kernel_spmd(nc, in_maps, core_ids=list(range(N_CORES)))
    return assemble_output(res.results)
